# revision 15
# baseline (speedup 1.0000x reference)
"""
MoE-routing kernel for Trainium2 (8 NeuronCores, SPMD via bass).

Computation (matches the reference):
  attended[b, c] = sum_hw((mask[b, hw] + 1e-10) * feat[b, c, hw]) / sum_hw(mask[b, hw] + 1e-10)
  out[b, a]      = attended[b, :] @ W[inst[b], a, :] + bias[inst[b], a]

Strategy: split the channel dim C=2048 into 8 shards of 256 (one per core).
Each core computes a partial contraction over its channel shard for ALL 256
samples; the host sums the 8 partials.  The batch is sorted by expert on the
host (static routing baked into the compiled program), so each expert's
samples form a contiguous group of stationary columns for the grouped GEMM.

Per core:
  phase 1 (pooling): for each sample, PE broadcasts the mask row to 128
    partitions (K=1 matmul), DVE does a fused multiply+reduce
    (tensor_tensor_reduce) against the feature tile -> one column of
    attended^T per (sample, c-tile).  Unnormalized (raw mask).
  phase 2 (grouped GEMM): per expert group, stationary = attended^T columns
    of the group, moving = W^T [c, a] chunks streamed from HBM (float32r ->
    full PE rate).  An extra K=1 matmul accumulates msum[b] * bias[e, a]
    into PSUM; eviction multiplies rows by 1/msum[b] (per-partition scalar),
    which normalizes the pooled features and leaves bias intact.
"""

import sys

if "/opt/trn_rl_repo" not in sys.path:
    sys.path.insert(0, "/opt/trn_rl_repo")

import numpy as np

import concourse.bass as bass
import concourse.mybir as mybir
import concourse.tile as tile
from concourse import bacc
from concourse import bass_utils
from concourse.masks import make_identity

# Problem constants (hardcoded; kernel.py must be self-contained)
B = 256          # batch
C = 2048         # channels
HW = 196         # spatial positions (14*14)
E = 16           # experts
A = 3000         # answers
NCORES = 8
CS = C // NCORES  # channel shard per core = 256
P = 128
KT = CS // P      # k-tiles per core = 2
MROW_BATCH = 16   # samples per partition-0 mask-row tile
HWP = 256         # padded mask row width (f32r wants moving free >= 256)
CHUNKS = [(c0, min(512, A - c0)) for c0 in range(0, A, 512)]

F32 = mybir.dt.float32
F32R = mybir.dt.float32r


def _make_groups(counts):
    """[(gstart_in_sorted_order, gsz, expert)] with gsz <= 128."""
    groups = []
    start = 0
    for e in range(E):
        n = int(counts[e])
        g0 = start
        while n > 0:
            gsz = min(n, P)
            groups.append((g0, gsz, e))
            g0 += gsz
            n -= gsz
        start += int(counts[e])
    return groups


def build_program(groups, loop_n=1, do_pool=True, do_mm=True, do_evict=True, pool_mode='full'):
    """Build + compile the per-core Bass program (identical on all cores)."""
    nc = bacc.Bacc("TRN2", target_bir_lowering=False, debug=False,
                   num_devices=NCORES)

    feat_d = nc.dram_tensor("feat", [B, CS, HW], F32, kind="ExternalInput").ap()
    mask_d = nc.dram_tensor("mask", [B, HW], F32, kind="ExternalInput").ap()
    wt_d = nc.dram_tensor("wt", [E, CS, A], F32R, kind="ExternalInput").ap()
    bias_d = nc.dram_tensor("bias", [1, E * A], F32R, kind="ExternalInput").ap()
    part_d = nc.dram_tensor("part", [B, A], F32, kind="ExternalOutput").ap()

    import contextlib
    with tile.TileContext(nc) as tc:
        loop_ctx = tc.For_i(0, loop_n, 1) if loop_n > 1 else contextlib.nullcontext()
        with (
            loop_ctx,
            tc.tile_pool(name="persist", bufs=1) as pp,
            tc.tile_pool(name="feat", bufs=6) as fp,
            tc.tile_pool(name="mrow", bufs=3) as mrp,
            tc.tile_pool(name="wt", bufs=6) as wtp,
            tc.tile_pool(name="bias", bufs=2) as bp,
            tc.tile_pool(name="outs", bufs=4) as op,
            tc.tile_pool(name="bcast", bufs=4) as pbc,
            tc.tile_pool(name="ps_mm", bufs=3, space="PSUM") as pmm,
            tc.tile_pool(name="ps_sm", bufs=2, space="PSUM") as psm,
        ):
            # ---- constants ----
            ident = pp.tile([P, P], F32, tag="ident")
            make_identity(nc, ident)
            ones32 = pp.tile([1, 1], F32, tag="ones32")
            nc.vector.memset(ones32, 1.0)
            dummy = pp.tile([P, 1], F32, tag="dummy")

            # ---- mask: per-sample sums and reciprocals ----
            mbp = pp.tile([P, KT, HW], F32, tag="mbp")
            nc.sync.dma_start(mbp, mask_d.rearrange("(t p) f -> p t f", p=P))
            msum = pp.tile([P, KT], F32, tag="msum")
            nc.vector.tensor_reduce(msum, mbp,
                                    axis=mybir.AxisListType.X,
                                    op=mybir.AluOpType.add)
            nc.vector.tensor_scalar_add(msum, msum, HW * 1e-10)

            # msum as a partition-0 row [1, B] (exact fp32 extraction matmuls)
            msum_row = pp.tile([1, B], F32R, tag="msum_row")
            for t in range(KT):
                pt = psm.tile([1, P], F32, name="pt_row", tag="pt")
                nc.tensor.matmul(pt, lhsT=msum[:, t:t + 1], rhs=ident,
                                 start=True, stop=True)
                nc.vector.tensor_copy(msum_row[0:1, t * P:(t + 1) * P], pt)
            recip_row = pp.tile([1, B], F32, tag="recip_row")
            nc.vector.reciprocal(recip_row, msum_row)

            # per-group reciprocals at partition base 0: rg[r] = 1/msum[g0+r]
            rgrps = []
            for gi, (g0, gsz, e) in enumerate(groups):
                rg = pp.tile([P, 1], F32, tag=f"rgrp{gi}", name=f"rgrp{gi}")
                pt = psm.tile([P, 1], F32, name="pt_col", tag="pt")
                nc.tensor.matmul(pt[:gsz], lhsT=recip_row[0:1, g0:g0 + gsz],
                                 rhs=ones32[0:1, 0:1], start=True, stop=True)
                nc.vector.tensor_copy(rg[:gsz], pt[:gsz])
                rgrps.append(rg)

            # mask rows on partition 0 for the PE broadcast (batched loads)
            mrows = []
            for mb in range(B // MROW_BATCH):
                mt = mrp.tile([1, MROW_BATCH, HW], F32, tag="mrow")
                nc.sync.dma_start(
                    mt, mask_d[mb * MROW_BATCH:(mb + 1) * MROW_BATCH, :]
                    .rearrange("(o s) f -> o s f", o=1))
                mrows.append(mt)

            # attended^T tiles, one per group: [128 c, KT, gsz]
            atts = [pp.tile([P, KT, gsz], F32R, tag=f"att{gi}", name=f"att{gi}")
                    for gi, (g0, gsz, e) in enumerate(groups)]
            if not do_pool:
                for att in atts:
                    nc.gpsimd.memset(att.bitcast(F32), 0.0)

            # ---- phase 1: masked pooling, one sample at a time ----
            sample_group = {}
            for gi, (g0, gsz, e) in enumerate(groups):
                for s in range(g0, g0 + gsz):
                    sample_group[s] = (gi, s - g0)
            for s in range(B):
                gi, pos = sample_group[s]
                ft = fp.tile([P, KT, HW], F32, tag="feat")
                nc.sync.dma_start(ft, feat_d[s].rearrange("(t p) f -> p t f", p=P))
                if do_pool:
                    bc = pbc.tile([P, HW], F32, name="bc")
                    if pool_mode in ("full", "bconly"):
                        nc.gpsimd.partition_broadcast(
                            bc, mrows[s // MROW_BATCH][0:1, s % MROW_BATCH])
                    if pool_mode == "full":
                        in1s = [bc] * KT
                    elif pool_mode == "sttsbuf":
                        in1s = [ft[:, t] for t in range(KT)]
                    else:
                        in1s = None
                    if in1s is not None:
                        for t in range(KT):
                            nc.vector.scalar_tensor_tensor(
                                dummy.broadcast_to([P, HW]),
                                ft[:, t], 1.0, in1s[t],
                                op0=mybir.AluOpType.mult, op1=mybir.AluOpType.mult,
                                accum_out=atts[gi][:, t, pos:pos + 1])

            # ---- phase 2: grouped GEMM over answer chunks ----
            bias_tiles = {}
            for gi, (g0, gsz, e) in enumerate(groups):
                if e not in bias_tiles:
                    bt = bp.tile([1, A], F32R, tag="bias")
                    nc.sync.dma_start(bt, bias_d[0:1, e * A:(e + 1) * A])
                    bias_tiles[e] = bt
                bt = bias_tiles[e]
                att = atts[gi]
                for (c0, cw) in CHUNKS:
                    wt = wtp.tile([P, KT, cw], F32R, tag="wt")
                    nc.sync.dma_start(
                        wt, wt_d[e].rearrange("(t p) a -> p t a", p=P)[:, :, c0:c0 + cw])
                    ot = op.tile([P, 512], F32, tag="out")
                    if not do_mm:
                        nc.gpsimd.memset(ot[:gsz, :cw], 0.0)
                    if do_mm:
                        ps = pmm.tile([P, 512], F32, name="ps")
                        for t in range(KT):
                            nc.tensor.matmul(
                                ps[:gsz, :cw],
                                lhsT=att[:, t],
                                rhs=wt[:, t],
                                start=(t == 0), stop=False)
                        nc.tensor.matmul(
                            ps[:gsz, :cw],
                            lhsT=msum_row[0:1, g0:g0 + gsz],
                            rhs=bt[0:1, c0:c0 + cw],
                            start=False, stop=True)
                        if do_evict:
                            nc.vector.tensor_scalar_mul(ot[:gsz, :cw], ps[:gsz, :cw],
                                                        rgrps[gi][:gsz])
                        else:
                            nc.gpsimd.memset(ot[:gsz, :cw], 0.0)
                    nc.sync.dma_start(part_d[g0:g0 + gsz, c0:c0 + cw],
                                      ot[:gsz, :cw])

    nc.compile()
    return nc


_PROGRAM_CACHE = {}


def _get_program(groups):
    key = tuple(groups)
    if key not in _PROGRAM_CACHE:
        _PROGRAM_CACHE[key] = build_program(groups)
    return _PROGRAM_CACHE[key]


def make_in_maps(mask, features, W, b, inst):
    """Host-side routing + sharding.  Returns (in_maps, perm, groups)."""
    inst_np = np.asarray(inst)
    perm = np.argsort(inst_np, kind="stable")
    counts = np.bincount(inst_np.astype(np.int64), minlength=E)
    groups = _make_groups(counts)

    mask_pad = np.ascontiguousarray(np.asarray(mask, np.float32).reshape(B, HW)[perm])

    feat = np.asarray(features, np.float32).reshape(B, C, HW)[perm]
    Wf = np.asarray(W, np.float32)
    bias_row = np.asarray(b, np.float32).reshape(1, E * A)
    zero_bias = np.zeros_like(bias_row)

    in_maps = []
    for k in range(NCORES):
        sl = slice(k * CS, (k + 1) * CS)
        feat_k = np.ascontiguousarray(feat[:, sl])
        wt_k = np.ascontiguousarray(Wf[:, :, sl].transpose(0, 2, 1))
        in_maps.append({
            "feat": feat_k,
            "mask": mask_pad,
            "wt": wt_k,
            "bias": bias_row if k == 0 else zero_bias,
        })
    return in_maps, perm, groups


def postprocess(results, perm):
    part = np.zeros((B, A), np.float32)
    for r in results:
        part += r["part"]
    out = np.empty((B, A), np.float32)
    out[perm] = part
    return out


def kernel(mask, features, W, b, inst):
    in_maps, perm, groups = make_in_maps(mask, features, W, b, inst)
    nc = _get_program(groups)
    res = bass_utils.run_bass_kernel_spmd(nc, in_maps, core_ids=list(range(NCORES)))
    return postprocess(res.results, perm)
